# revision 1
# baseline (speedup 1.0000x reference)
"""Trainium2 Bass kernel for nn_ConvTranBackbone (conv tokenizer + 4-layer
transformer encoder). Data-parallel over batch: 16 batch elems -> 8 cores x 2.

Self-contained: hardcodes shapes/sharding; host-folds BN/LN affines into
weights; runs one SPMD Bass program on cores 0-7 via run_bass_kernel_spmd.

Design notes:
- Residual stream kept feature-major ([d mod 128, d//128, token]) in fp32r.
- LayerNorm stats via ones-matmul partition reduction (broadcast built in).
- Attention computes transposed scores (j on partitions) so softmax's Z and
  the probs@V contraction both run as matmuls; the relative-position bias is
  accumulated into score PSUM via an identity matmul reading a host-built
  shifted "strip" (bf16); Z rides in ones-columns of the padded V lhsT.
- All matmuls fp32r (1 cycle/row at N>=512) except bias strips (bf16) and
  the final fp32 transpose.
"""
import sys
import math

sys.path.insert(0, '/opt/trn_rl_repo')

import numpy as np
import ml_dtypes

import concourse.bass as bass
import concourse.bacc as bacc
import concourse.mybir as mybir
import concourse.tile as tile
from concourse.bass_utils import run_bass_kernel_spmd

F32 = mybir.dt.float32
F32R = mybir.dt.float32r
BF16 = mybir.dt.bfloat16
AF = mybir.ActivationFunctionType
ALU = mybir.AluOpType

B, C_IN, S, D, H, L, FF = 16, 32, 512, 256, 8, 4, 1024
HD = D // H          # 32
EPS = 1e-5
NCORES = 8
BLOC = B // NCORES   # 2 batch elems per core
DB = 2               # d blocks of 128
TOK = BLOC * S       # 1024 tokens per core
IC = TOK // 512      # i-chunks of 512 tokens

TRACE = False
_CACHE = {}


# ---------------------------------------------------------------- host prep
def _pos_encoding():
    pos = np.arange(S, dtype=np.float32)[:, None]
    div = np.exp(np.arange(0, D, 2, dtype=np.float32) * (-math.log(10000.0) / D))
    scale = D / S
    pe = np.zeros((S, D), dtype=np.float32)
    pe[:, 0::2] = np.sin(pos * div * scale)
    pe[:, 1::2] = np.cos(pos * div * scale)
    return pe


def _prep(inp):
    f = lambda x: np.ascontiguousarray(np.asarray(x, np.float32))
    p = {}
    s1 = f(inp['bn1_g']) / np.sqrt(np.float32(1.0) + np.float32(EPS))
    b1c = f(inp['conv1_b']) * s1 + f(inp['bn1_b'])
    s2 = f(inp['bn2_g']) / np.sqrt(np.float32(1.0) + np.float32(EPS))
    b2c = f(inp['conv2_b']) * s2 + f(inp['bn2_b'])
    cvec = np.zeros((128, DB, 4), np.float32)
    for db in range(DB):
        cvec[:, db, 0] = s1[db * 128:(db + 1) * 128]
        cvec[:, db, 1] = b1c[db * 128:(db + 1) * 128]
        cvec[:, db, 2] = s2[db * 128:(db + 1) * 128]
        cvec[:, db, 3] = b2c[db * 128:(db + 1) * 128]
    p['cvec'] = cvec

    w1 = f(inp['conv1_w'])
    w1A = np.zeros((128, D), np.float32)
    for kk in range(4):
        w1A[32 * kk:32 * kk + 32, :] = w1[:, :, kk].T
    w1B = np.zeros((96, D), np.float32)
    for j in range(3):
        w1B[32 * j:32 * j + 32, :] = w1[:, :, 4 + j].T
    p['w1A'], p['w1B'] = w1A, np.ascontiguousarray(w1B)

    w2 = f(inp['conv2_w'])
    w2t = np.zeros((128, DB, 5, D), np.float32)
    for cb in range(DB):
        for k in range(5):
            w2t[:, cb, k, :] = w2[:, cb * 128:(cb + 1) * 128, k].T
    p['w2t'] = w2t

    pe = _pos_encoding()
    p['peT'] = np.ascontiguousarray(pe.T.reshape(DB, 128, S).transpose(1, 0, 2))

    sc = np.float32(HD ** -0.5)
    for l in range(L):
        g1, b1l = f(inp['ln1_g'][l]), f(inp['ln1_b'][l])
        g2, b2l = f(inp['ln2_g'][l]), f(inp['ln2_b'][l])
        wq = f(inp['wq'][l]) * sc
        wk, wv, wo = f(inp['wk'][l]), f(inp['wv'][l]), f(inp['wo'][l])
        wm = np.zeros((128, 4, DB, D), np.float32)
        for i, w in enumerate([g1[:, None] * wq, g1[:, None] * wk,
                               g1[:, None] * wv, wo]):
            for kb in range(DB):
                wm[:, i, kb, :] = w[kb * 128:(kb + 1) * 128, :]
        p[f'wqkvo{l}'] = wm
        p[f'bvb{l}'] = np.tile((b1l @ wv)[None, :], (128, 1)).astype(np.float32)
        w1f = f(inp['w1'][l])
        w1m = np.zeros((128, DB, FF), np.float32)
        w1e = g2[:, None] * w1f
        for kb in range(DB):
            w1m[:, kb, :] = w1e[kb * 128:(kb + 1) * 128, :]
        p[f'wff1{l}'] = w1m
        w2f = f(inp['w2'][l])
        w2m = np.zeros((128, 8, D), np.float32)
        for kb in range(8):
            w2m[:, kb, :] = w2f[kb * 128:(kb + 1) * 128, :]
        p[f'wff2{l}'] = w2m
        # per-partition bias pack: cols [bq(2), bk(2), bo(2), b2(2), b1(8)]
        pv = np.zeros((128, 16), np.float32)
        bq, bk = b1l @ wq, b1l @ wk
        bo, b2v = f(inp['bo'][l]), f(inp['b2'][l])
        b1e = b2l @ w1f + f(inp['b1'][l])
        for db in range(DB):
            pv[:, 0 + db] = bq[db * 128:(db + 1) * 128]
            pv[:, 2 + db] = bk[db * 128:(db + 1) * 128]
            pv[:, 4 + db] = bo[db * 128:(db + 1) * 128]
            pv[:, 6 + db] = b2v[db * 128:(db + 1) * 128]
        for fb in range(8):
            pv[:, 8 + fb] = b1e[fb * 128:(fb + 1) * 128]
        p[f'pvec{l}'] = pv
        tab = f(inp['bias_table'][l])            # [2S-1, H]
        st = np.zeros((128, H, 1024), np.float32)
        for pp in range(128):
            hi = min(1024, pp + 2 * S - 1)
            st[pp, :, pp:hi] = tab[0:hi - pp, :].T
        p[f'strip{l}'] = st.astype(ml_dtypes.bfloat16)
    fvec = np.zeros((128, 4), np.float32)
    for db in range(DB):
        fvec[:, 0 + db] = f(inp['fn_g'])[db * 128:(db + 1) * 128]
        fvec[:, 2 + db] = f(inp['fn_b'])[db * 128:(db + 1) * 128]
    p['fvec'] = fvec
    p['identb'] = np.eye(128, dtype=ml_dtypes.bfloat16)
    p['identf'] = np.eye(128, dtype=np.float32)
    p['identr'] = np.eye(128, dtype=np.float32)
    p['onesd'] = np.full((128, 128), 1.0 / 256.0, np.float32)
    p['zeros16'] = np.zeros((128, 16), np.float32)
    # vz slot template: per head slot h, Z-ones at cols [64*(h%2)+32, +32)
    vzt = np.zeros((128, 4, 4, 128), np.float32)
    for h in range(4):
        par = h % 2
        vzt[:, :, h, 64 * par + 32:64 * par + 64] = 1.0
    p['vztmpl'] = vzt
    return p


# ---------------------------------------------------------------- device build
def _build(repeat=1, upto='full'):
    nc = bacc.Bacc()
    din = {}

    def dinp(name, shape, dt=F32R):
        din[name] = nc.dram_tensor(name, list(shape), dt, kind='ExternalInput')
        return din[name]

    x = dinp('x', [BLOC, C_IN, S])
    w1A = dinp('w1A', [128, D])
    w1B = dinp('w1B', [96, D])
    w2t = dinp('w2t', [128, DB, 5, D])
    cvec = dinp('cvec', [128, DB, 4], F32)
    peT = dinp('peT', [128, DB, S])
    identb = dinp('identb', [128, 128], BF16)
    identf = dinp('identf', [128, 128], F32)
    identr = dinp('identr', [128, 128], F32R)
    onesd = dinp('onesd', [128, 128], F32R)
    zeros16 = dinp('zeros16', [128, 16], F32R)
    vztmpl = dinp('vztmpl', [128, 4, 4, 128], F32R)
    fvec = dinp('fvec', [128, 4], F32)
    for l in range(L):
        dinp(f'wqkvo{l}', [128, 4, DB, D])
        dinp(f'wff1{l}', [128, DB, FF])
        dinp(f'wff2{l}', [128, 8, D])
        dinp(f'bvb{l}', [128, D])
        dinp(f'pvec{l}', [128, 16], F32)
        dinp(f'strip{l}', [128, H, 1024], BF16)
    out = nc.dram_tensor('out', [BLOC, S, D], F32, kind='ExternalOutput')

    tc_cm = tile.TileContext(nc)
    tc = tc_cm.__enter__()
    cst = tc.alloc_tile_pool(name='cst', bufs=1)
    wp = tc.alloc_tile_pool(name='wp', bufs=2)
    wp1 = tc.alloc_tile_pool(name='wp1', bufs=1)
    ap = tc.alloc_tile_pool(name='ap', bufs=1)
    tp = tc.alloc_tile_pool(name='tp', bufs=2)
    tp1 = tc.alloc_tile_pool(name='tp1', bufs=1)
    vzp = tc.alloc_tile_pool(name='vzp', bufs=2)
    prp = tc.alloc_tile_pool(name='prp', bufs=2)
    h1p = tc.alloc_tile_pool(name='h1p', bufs=8)
    ps_mm = tc.alloc_tile_pool(name='ps_mm', bufs=2, space='PSUM')
    ps_sc = tc.alloc_tile_pool(name='ps_sc', bufs=2, space='PSUM')
    ps_cz = tc.alloc_tile_pool(name='ps_cz', bufs=4, space='PSUM')

    # ---- consts
    identb_s = cst.tile([128, 128], BF16)
    nc.sync.dma_start(identb_s[:], identb[:])
    identf_s = cst.tile([128, 128], F32)
    nc.sync.dma_start(identf_s[:], identf[:])
    identr_s = cst.tile([128, 128], F32R)
    nc.sync.dma_start(identr_s[:], identr[:])
    onesd_s = cst.tile([128, 128], F32R)
    nc.sync.dma_start(onesd_s[:], onesd[:])
    z16_s = cst.tile([128, 16], F32R)
    nc.sync.dma_start(z16_s[:], zeros16[:])
    cvec_s = cst.tile([128, DB, 4], F32)
    nc.sync.dma_start(cvec_s[:], cvec[:])
    fvec_s = cst.tile([128, 4], F32)
    nc.sync.dma_start(fvec_s[:], fvec[:])
    peT_s = cst.tile([128, DB, S], F32R)
    nc.sync.dma_start(peT_s[:], peT[:])
    eps_s = cst.tile([128, 1], F32)
    nc.vector.memset(eps_s[:], EPS)
    w1A_s = cst.tile([128, D], F32R)
    nc.sync.dma_start(w1A_s[:], w1A[:])
    w1B_s = cst.tile([96, D], F32R)
    nc.sync.dma_start(w1B_s[:], w1B[:])
    w2t_s = cst.tile([128, DB, 5, D], F32R)
    nc.sync.dma_start(w2t_s[:], w2t[:])

    # vz slot templates (zeros + Z-ones). v columns are rewritten per use;
    # the static template regions persist across pool-slot reuse.
    for i in range(2):
        vzt_t = vzp.tile([128, 4, 4, 128], F32R, tag='vz', name=f'vzi{i}')
        nc.sync.dma_start(vzt_t[:], vztmpl[:])

    # persistent residual stream, feature-major [d mod 128, db, token]
    rt = ap.tile([128, DB, TOK], F32R)

    def emit_body(R):
        # ---------------- conv tokenizer
        for b in range(BLOC):
            X4 = h1p.tile([128, 512], F32R, tag='h1', name=f'{R}x4_{b}')
            nc.sync.dma_start(X4[0:32, 3:512], x[b, :, 0:509])
            nc.sync.dma_start(X4[32:64, 2:512], x[b, :, 0:510])
            nc.sync.dma_start(X4[64:96, 1:512], x[b, :, 0:511])
            nc.sync.dma_start(X4[96:128, 0:512], x[b, :, 0:512])
            nc.sync.dma_start(X4[0:32, 0:3], z16_s[0:32, 0:3])
            nc.sync.dma_start(X4[32:64, 0:2], z16_s[32:64, 0:2])
            nc.sync.dma_start(X4[64:96, 0:1], z16_s[64:96, 0:1])
            X3 = h1p.tile([128, 512], F32R, tag='h1', name=f'{R}x3_{b}')
            nc.sync.dma_start(X3[0:32, 0:511], x[b, :, 1:512])
            nc.sync.dma_start(X3[32:64, 0:510], x[b, :, 2:512])
            nc.sync.dma_start(X3[64:96, 0:509], x[b, :, 3:512])
            nc.sync.dma_start(X3[0:32, 511:512], z16_s[0:32, 0:1])
            nc.sync.dma_start(X3[32:64, 510:512], z16_s[32:64, 0:2])
            nc.sync.dma_start(X3[64:96, 509:512], z16_s[64:96, 0:3])
            hp = tp.tile([128, DB, 516], F32R, tag='xn', name=f'{R}hp_{b}')
            for dc in range(DB):
                psc = ps_mm.tile([128, 512], F32, tag='mm', name=f'{R}c1_{b}_{dc}')
                nc.tensor.matmul(psc[:], w1A_s[:, dc * 128:(dc + 1) * 128],
                                 X4[:], start=True, stop=False,
                                 skip_group_check=True)
                nc.tensor.matmul(psc[:], w1B_s[:, dc * 128:(dc + 1) * 128],
                                 X3[0:96, :], start=False, stop=True,
                                 skip_group_check=True)
                nc.sync.dma_start(hp[:, dc, 0:2], z16_s[:, 0:2])
                nc.sync.dma_start(hp[:, dc, 514:516], z16_s[:, 0:2])
                nc.scalar.activation(hp[:, dc, 2:514], psc[:], AF.Gelu,
                                     bias=cvec_s[:, dc, 1:2],
                                     scale=cvec_s[:, dc, 0:1])
            for dc in range(DB):
                ps2 = ps_mm.tile([128, 512], F32, tag='mm', name=f'{R}c2_{b}_{dc}')
                for cb in range(DB):
                    for k in range(5):
                        nc.tensor.matmul(
                            ps2[:], w2t_s[:, cb, k, dc * 128:(dc + 1) * 128],
                            hp[:, cb, k:k + 512],
                            start=(cb == 0 and k == 0),
                            stop=(cb == 1 and k == 4), skip_group_check=True)
                tg = h1p.tile([128, 512], F32R, tag='h1', name=f'{R}tg_{b}_{dc}')
                nc.scalar.activation(tg[:], ps2[:], AF.Gelu,
                                     bias=cvec_s[:, dc, 3:4],
                                     scale=cvec_s[:, dc, 2:3])
                nc.vector.tensor_add(rt[:, dc, b * S:(b + 1) * S],
                                     tg[:], peT_s[:, dc, :])

        # ---------------- layernorm helper (feature-major, stats via ones-MM)
        def layernorm(xn_t, tag):
            outs = []
            for ic in range(IC):
                sl = slice(ic * 512, (ic + 1) * 512)
                sq = tp1.tile([128, DB, 512], F32R, tag='sq', name=f'{R}sq_{tag}{ic}')
                nc.gpsimd.tensor_tensor(sq[:], rt[:, :, sl], rt[:, :, sl], ALU.mult)
                mu_ps = ps_mm.tile([128, 512], F32, tag='mm', name=f'{R}mu_{tag}{ic}')
                s2_ps = ps_mm.tile([128, 512], F32, tag='mm', name=f'{R}s2_{tag}{ic}')
                for db in range(DB):
                    nc.tensor.matmul(mu_ps[:], onesd_s[:], rt[:, db, sl],
                                     start=(db == 0), stop=(db == 1),
                                     skip_group_check=True)
                for db in range(DB):
                    nc.tensor.matmul(s2_ps[:], onesd_s[:], sq[:, db, :],
                                     start=(db == 0), stop=(db == 1),
                                     skip_group_check=True)
                mu_sb = tp1.tile([128, 512], F32, tag='mu', name=f'{R}mu2_{tag}{ic}')
                nc.vector.tensor_copy(mu_sb[:], mu_ps[:])
                m2 = tp1.tile([128, 512], F32, tag='m2', name=f'{R}m2_{tag}{ic}')
                nc.gpsimd.tensor_tensor(m2[:], mu_sb[:], mu_sb[:], ALU.mult)
                var = tp1.tile([128, 512], F32, tag='var', name=f'{R}var_{tag}{ic}')
                nc.vector.tensor_sub(var[:], s2_ps[:], m2[:])
                sd = tp1.tile([128, 512], F32, tag='sd', name=f'{R}sd_{tag}{ic}')
                nc.scalar.activation(sd[:], var[:], AF.Sqrt, bias=eps_s[:])
                rstd = tp.tile([128, 512], F32, tag='rstd', name=f'{R}rs_{tag}{ic}')
                nc.vector.reciprocal(rstd[:], sd[:])
                nm = tp.tile([128, 512], F32, tag='nm', name=f'{R}nm_{tag}{ic}')
                nc.vector.scalar_tensor_tensor(nm[:], mu_sb[:], -1.0, rstd[:],
                                               ALU.mult, ALU.mult)
                if xn_t is not None:
                    for db in range(DB):
                        t1 = tp1.tile([128, 512], F32, tag='t1',
                                     name=f'{R}t1_{tag}{ic}{db}')
                        nc.gpsimd.tensor_tensor(t1[:], rt[:, db, sl], rstd[:],
                                                ALU.mult)
                        nc.vector.tensor_add(xn_t[:, db, sl], t1[:], nm[:])
                outs.append((rstd, nm))
            return outs

        # ---------------- transformer layers
        if upto == 'conv':
            return
        for l in range(L):
            wqkvo_s = wp.tile([128, 4, DB, D], F32R, tag='wqkvo', name=f'{R}wm{l}')
            nc.sync.dma_start(wqkvo_s[:], din[f'wqkvo{l}'][:])
            wff1_s = wp.tile([128, DB, FF], F32R, tag='wff1', name=f'{R}w1{l}')
            nc.sync.dma_start(wff1_s[:], din[f'wff1{l}'][:])
            wff2_s = wp1.tile([128, 8, D], F32R, tag='wff2', name=f'{R}w2{l}')
            nc.sync.dma_start(wff2_s[:], din[f'wff2{l}'][:])
            bvb_s = wp.tile([128, D], F32R, tag='bvb', name=f'{R}bv{l}')
            nc.sync.dma_start(bvb_s[:], din[f'bvb{l}'][:])
            pvec_s = wp.tile([128, 16], F32, tag='pvec', name=f'{R}pv{l}')
            nc.sync.dma_start(pvec_s[:], din[f'pvec{l}'][:])
            strip_s = wp1.tile([128, H, 1024], BF16, tag='strip', name=f'{R}st{l}')
            nc.sync.dma_start(strip_s[:], din[f'strip{l}'][:])

            # LN1 -> xn
            xn = tp.tile([128, DB, TOK], F32R, tag='xn', name=f'{R}xn{l}')
            layernorm(xn, f'a{l}')

            # q/k projections (feature-major, per-dout bias folded into evac)
            qT = ap.tile([128, DB, TOK], F32R, tag='qT', name=f'{R}qT{l}')
            kT = ap.tile([128, DB, TOK], F32R, tag='kT', name=f'{R}kT{l}')
            for mat, (dst, bc) in enumerate([(qT, 0), (kT, 2)]):
                for mb in range(DB):
                    for ic in range(IC):
                        sl = slice(ic * 512, (ic + 1) * 512)
                        ps = ps_mm.tile([128, 512], F32, tag='mm',
                                        name=f'{R}qk{l}{mat}{mb}{ic}')
                        for kb in range(DB):
                            nc.tensor.matmul(
                                ps[:],
                                wqkvo_s[:, mat, kb, mb * 128:(mb + 1) * 128],
                                xn[:, kb, sl], start=(kb == 0), stop=(kb == 1),
                                skip_group_check=True)
                        nc.vector.tensor_scalar(
                            dst[:, mb, sl], ps[:],
                            pvec_s[:, bc + mb:bc + mb + 1], None, ALU.add)

            if upto == 'qkvv':
                continue
            ctxT = ap.tile([128, DB, TOK], F32R, tag='ctxT', name=f'{R}cx{l}')
            for b in range(BLOC):
                # v projection, scattered into padded vz slots for both hblks
                vzs = []
                for hb in range(2):
                    vz = vzp.tile([128, 4, 4, 128], F32R, tag='vz',
                                  name=f'{R}vz{l}{b}{hb}')
                    vzs.append(vz)
                for jc in range(4):
                    vp = ps_mm.tile([128, 256], F32, tag='mm', name=f'{R}v{l}{b}{jc}')
                    nc.tensor.matmul(vp[:], identr_s[:], bvb_s[:],
                                     start=True, stop=False, skip_group_check=True)
                    for kb in range(DB):
                        nc.tensor.matmul(
                            vp[:],
                            xn[:, kb, b * S + jc * 128:b * S + (jc + 1) * 128],
                            wqkvo_s[:, 2, kb, :], start=False, stop=(kb == 1),
                            skip_group_check=True)
                    vp_r = vp.rearrange('p (hb he pc) -> p hb he pc', hb=2, pc=64)
                    for hb in range(2):
                        vz_r = vzs[hb].rearrange(
                            'p jc (he two) m -> p jc he two m', two=2)
                        for par in range(2):
                            nc.vector.tensor_copy(
                                vz_r[:, jc, :, par, 64 * par:64 * par + 32],
                                vp_r[:, hb, :, 32 * par:32 * par + 32])
                # attention per head-block: scoresT -> exp -> ctx+Z matmuls
                for hb in range(2):
                    banks = [ps_cz.tile([128, 512], F32, tag='cz',
                                        name=f'{R}cb{l}{b}{hb}{pb}') for pb in range(2)]
                    for jc in range(4):
                        probs = []
                        for hh in range(4):
                            scp = ps_sc.tile([128, 512], F32, tag='sc',
                                             name=f'{R}sc{l}{b}{hb}{jc}{hh}')
                            nc.tensor.matmul(
                                scp[:], identb_s[:],
                                strip_s[:, 4 * hb + hh,
                                        511 - jc * 128:1023 - jc * 128],
                                start=True, stop=False, skip_group_check=True)
                            nc.tensor.matmul(
                                scp[:],
                                kT[32 * hh:32 * hh + 32, hb,
                                   b * S + jc * 128:b * S + (jc + 1) * 128],
                                qT[32 * hh:32 * hh + 32, hb, b * S:(b + 1) * S],
                                start=False, stop=True,
                                tile_position=(32 * hh, 0), skip_group_check=True)
                            pr = prp.tile([128, 512], F32R, tag=f'p{hh}',
                                          name=f'{R}pr{l}{b}{hb}{jc}{hh}')
                            nc.scalar.activation(pr[:], scp[:], AF.Exp)
                            probs.append(pr)
                        for hh in range(4):
                            nc.tensor.matmul(
                                banks[hh // 2][:], vzs[hb][:, jc, hh, :],
                                probs[hh][:],
                                start=(jc == 0 and hh % 2 == 0),
                                stop=(jc == 3 and hh % 2 == 1),
                                skip_group_check=True)
                    # normalize + assemble ctxT rows [32h, +32)
                    for pb in range(2):
                        rec = tp1.tile([128, 512], F32, tag='rec',
                                      name=f'{R}rc{l}{b}{hb}{pb}')
                        nc.vector.reciprocal(rec[:], banks[pb][:])
                        zrec = tp1.tile([128, 512], F32, tag='zrec',
                                       name=f'{R}zr{l}{b}{hb}{pb}')
                        nc.sync.dma_start(zrec[0:32, :], rec[32:64, :])
                        nc.sync.dma_start(zrec[64:96, :], rec[96:128, :])
                        ctxn = tp1.tile([128, 512], F32R, tag='ctxn',
                                       name=f'{R}cn{l}{b}{hb}{pb}')
                        nc.vector.tensor_mul(ctxn[:], banks[pb][:], zrec[:])
                        base = 64 * pb
                        nc.sync.dma_start(
                            ctxT[base:base + 32, hb, b * S:(b + 1) * S],
                            ctxn[0:32, :])
                        nc.sync.dma_start(
                            ctxT[base + 32:base + 64, hb, b * S:(b + 1) * S],
                            ctxn[64:96, :])

            if upto == 'attn':
                continue
            # out-projection + residual (bias via fused scalar_tensor_tensor)
            for mb in range(DB):
                for ic in range(IC):
                    sl = slice(ic * 512, (ic + 1) * 512)
                    ps = ps_mm.tile([128, 512], F32, tag='mm',
                                    name=f'{R}op{l}{mb}{ic}')
                    for kb in range(DB):
                        nc.tensor.matmul(
                            ps[:], wqkvo_s[:, 3, kb, mb * 128:(mb + 1) * 128],
                            ctxT[:, kb, sl], start=(kb == 0), stop=(kb == 1),
                            skip_group_check=True)
                    nc.vector.scalar_tensor_tensor(
                        rt[:, mb, sl], ps[:], pvec_s[:, 4 + mb:5 + mb],
                        rt[:, mb, sl], ALU.add, ALU.add)

            # LN2 -> xn2
            xn2 = tp.tile([128, DB, TOK], F32R, tag='xn', name=f'{R}xn2_{l}')
            layernorm(xn2, f'f{l}')

            # FFN
            for ic in range(IC):
                sl = slice(ic * 512, (ic + 1) * 512)
                h1s = []
                for fb in range(8):
                    ps = ps_mm.tile([128, 512], F32, tag='mm', name=f'{R}h1{l}{ic}{fb}')
                    for kb in range(DB):
                        nc.tensor.matmul(
                            ps[:], wff1_s[:, kb, fb * 128:(fb + 1) * 128],
                            xn2[:, kb, sl], start=(kb == 0), stop=(kb == 1),
                            skip_group_check=True)
                    h1t = h1p.tile([128, 512], F32R, tag='h1', name=f'{R}h1t{l}{ic}{fb}')
                    nc.scalar.activation(h1t[:], ps[:], AF.Gelu,
                                         bias=pvec_s[:, 8 + fb:9 + fb])
                    h1s.append(h1t)
                for db in range(DB):
                    ps = ps_cz.tile([128, 512], F32, tag='cz', name=f'{R}h2{l}{ic}{db}')
                    for fb in range(8):
                        nc.tensor.matmul(
                            ps[:], wff2_s[:, fb, db * 128:(db + 1) * 128],
                            h1s[fb][:], start=(fb == 0), stop=(fb == 7),
                            skip_group_check=True)
                    nc.vector.scalar_tensor_tensor(
                        rt[:, db, sl], ps[:], pvec_s[:, 6 + db:7 + db],
                        rt[:, db, sl], ALU.add, ALU.add)

        if upto != 'full':
            sink = h1p.tile([128, 256], F32, tag='h1', name=f'{R}sink')
            nc.vector.tensor_copy(sink[:], rt[:, 0, 0:256])
            nc.sync.dma_start(out[0, 0:128, :], sink[:])
            return
        # ---------------- final LN (+affine) and transpose to token-major
        fin = tp.tile([128, DB, TOK], F32, tag='xn', name=f'{R}fin')
        stats = layernorm(None, 'fin')
        for ic in range(IC):
            sl = slice(ic * 512, (ic + 1) * 512)
            rstd, nm = stats[ic]
            for db in range(DB):
                rstd_g = tp1.tile([128, 512], F32, tag='rstd_g',
                                 name=f'{R}rg{ic}{db}')
                nc.vector.tensor_scalar(rstd_g[:], rstd[:],
                                        fvec_s[:, 0 + db:1 + db], None, ALU.mult)
                nm_gb = tp1.tile([128, 512], F32, tag='nm_gb', name=f'{R}ng{ic}{db}')
                nc.vector.tensor_scalar(nm_gb[:], nm[:],
                                        fvec_s[:, 0 + db:1 + db],
                                        fvec_s[:, 2 + db:3 + db],
                                        ALU.mult, ALU.add)
                t1 = tp1.tile([128, 512], F32, tag='t1', name=f'{R}ft1{ic}{db}')
                nc.gpsimd.tensor_tensor(t1[:], rt[:, db, sl], rstd_g[:], ALU.mult)
                nc.vector.tensor_add(fin[:, db, sl], t1[:], nm_gb[:])
        for b in range(BLOC):
            for jc in range(4):
                tc_sl = slice(b * S + jc * 128, b * S + (jc + 1) * 128)
                pst = ps_mm.tile([128, 256], F32, tag='mm', name=f'{R}tr{b}{jc}')
                for db in range(DB):
                    nc.tensor.transpose(pst[:, db * 128:(db + 1) * 128],
                                        fin[:, db, tc_sl], identf_s[:])
                osb = h1p.tile([128, 256], F32, tag='h1', name=f'{R}ot{b}{jc}')
                nc.vector.tensor_copy(osb[:], pst[:])
                nc.sync.dma_start(out[b, jc * 128:(jc + 1) * 128, :], osb[:])


    for _rep in range(repeat):
        emit_body(f'r{_rep}_')

    for pool in [ps_cz, ps_sc, ps_mm, h1p, prp, vzp, tp1, tp, ap, wp1, wp, cst]:
        pool.release()
    tc_cm.__exit__(None, None, None)
    nc.finalize()
    return nc


# ---------------------------------------------------------------- entry point
def kernel(**inputs):
    p = _prep(inputs)
    if 'nc' not in _CACHE:
        _CACHE['nc'] = _build()
    nc = _CACHE['nc']
    x = np.ascontiguousarray(np.asarray(inputs['x'], np.float32))
    in_maps = []
    for c in range(NCORES):
        m = dict(p)
        m['x'] = np.ascontiguousarray(x[c * BLOC:(c + 1) * BLOC])
        in_maps.append(m)
    res = run_bass_kernel_spmd(nc, in_maps, core_ids=list(range(NCORES)),
                               trace=TRACE)
    out = np.concatenate([r['out'] for r in res.results], axis=0)
    kernel.last_results = res
    return np.ascontiguousarray(out.astype(np.float32))



# revision 13
# speedup vs baseline: 1.2468x; 1.2468x over previous
"""Trainium2 Bass kernel for nn_ConvTranBackbone (conv tokenizer + 4-layer
transformer encoder). Data-parallel over batch: 16 batch elems -> 8 cores x 2.

Self-contained: hardcodes shapes/sharding; host-folds BN/LN affines into
weights; runs one SPMD Bass program on cores 0-7 via run_bass_kernel_spmd.

Design notes:
- Residual stream kept feature-major ([d mod 128, d//128, token]) in fp32r.
- LayerNorm stats via ones-matmul partition reduction (broadcast built in).
- Attention computes transposed scores (j on partitions) so softmax's Z and
  the probs@V contraction both run as matmuls; the relative-position bias is
  accumulated into score PSUM via an identity matmul reading a host-built
  shifted "strip" (bf16); Z rides in ones-columns of the padded V lhsT.
- All matmuls fp32r (1 cycle/row at N>=512) except bias strips (bf16) and
  the final fp32 transpose.
"""
import sys
import math

sys.path.insert(0, '/opt/trn_rl_repo')

import numpy as np
import ml_dtypes

import concourse.bass as bass
import concourse.bacc as bacc
import concourse.mybir as mybir
import concourse.tile as tile
from concourse.bass_utils import run_bass_kernel_spmd

F32 = mybir.dt.float32
F32R = mybir.dt.float32r
BF16 = mybir.dt.bfloat16
AF = mybir.ActivationFunctionType
ALU = mybir.AluOpType

B, C_IN, S, D, H, L, FF = 16, 32, 512, 256, 8, 4, 1024
HD = D // H          # 32
EPS = 1e-5
NCORES = 8
BLOC = B // NCORES   # 2 batch elems per core
DB = 2               # d blocks of 128
TOK = BLOC * S       # 1024 tokens per core
IC = TOK // 512      # i-chunks of 512 tokens

TRACE = False
_CACHE = {}


# ---------------------------------------------------------------- host prep
def _pos_encoding():
    pos = np.arange(S, dtype=np.float32)[:, None]
    div = np.exp(np.arange(0, D, 2, dtype=np.float32) * (-math.log(10000.0) / D))
    scale = D / S
    pe = np.zeros((S, D), dtype=np.float32)
    pe[:, 0::2] = np.sin(pos * div * scale)
    pe[:, 1::2] = np.cos(pos * div * scale)
    return pe


def _prep(inp):
    f = lambda x: np.ascontiguousarray(np.asarray(x, np.float32))
    p = {}
    s1 = f(inp['bn1_g']) / np.sqrt(np.float32(1.0) + np.float32(EPS))
    b1c = f(inp['conv1_b']) * s1 + f(inp['bn1_b'])
    s2 = f(inp['bn2_g']) / np.sqrt(np.float32(1.0) + np.float32(EPS))
    b2c = f(inp['conv2_b']) * s2 + f(inp['bn2_b'])
    cvec = np.zeros((128, DB, 4), np.float32)
    for db in range(DB):
        cvec[:, db, 0] = s1[db * 128:(db + 1) * 128]
        cvec[:, db, 1] = b1c[db * 128:(db + 1) * 128]
        cvec[:, db, 2] = s2[db * 128:(db + 1) * 128]
        cvec[:, db, 3] = b2c[db * 128:(db + 1) * 128]
    p['cvec'] = cvec

    w1 = f(inp['conv1_w'])
    w1A = np.zeros((128, D), np.float32)
    for kk in range(4):
        w1A[32 * kk:32 * kk + 32, :] = w1[:, :, kk].T
    w1B = np.zeros((96, D), np.float32)
    for j in range(3):
        w1B[32 * j:32 * j + 32, :] = w1[:, :, 4 + j].T
    p['w1A'], p['w1B'] = w1A, np.ascontiguousarray(w1B)

    w2 = f(inp['conv2_w'])
    w2t = np.zeros((128, DB, 5, D), np.float32)
    for cb in range(DB):
        for k in range(5):
            w2t[:, cb, k, :] = w2[:, cb * 128:(cb + 1) * 128, k].T
    p['w2t'] = w2t

    pe = _pos_encoding()
    p['peT'] = np.ascontiguousarray(pe.T.reshape(DB, 128, S).transpose(1, 0, 2))

    sc = np.float32(HD ** -0.5)
    for l in range(L):
        g1, b1l = f(inp['ln1_g'][l]), f(inp['ln1_b'][l])
        g2, b2l = f(inp['ln2_g'][l]), f(inp['ln2_b'][l])
        wq = f(inp['wq'][l]) * sc
        wk, wv, wo = f(inp['wk'][l]), f(inp['wv'][l]), f(inp['wo'][l])
        wm = np.zeros((128, 4, DB, D), np.float32)
        for i, w in enumerate([g1[:, None] * wq, g1[:, None] * wk,
                               g1[:, None] * wv, wo]):
            for kb in range(DB):
                wm[:, i, kb, :] = w[kb * 128:(kb + 1) * 128, :]
        p[f'wqkvo{l}'] = wm
        p[f'bvb{l}'] = np.tile((b1l @ wv)[None, :], (128, 1)).astype(np.float32)
        w1f = f(inp['w1'][l])
        w1m = np.zeros((128, DB, FF), np.float32)
        w1e = g2[:, None] * w1f
        for kb in range(DB):
            w1m[:, kb, :] = w1e[kb * 128:(kb + 1) * 128, :]
        p[f'wff1{l}'] = w1m
        w2f = f(inp['w2'][l])
        w2m = np.zeros((128, 8, D), np.float32)
        for kb in range(8):
            w2m[:, kb, :] = w2f[kb * 128:(kb + 1) * 128, :]
        p[f'wff2{l}'] = w2m
        # per-partition bias pack: cols [bq(2), bk(2), bo(2), b2(2), b1(8)]
        pv = np.zeros((128, 16), np.float32)
        bq, bk = b1l @ wq, b1l @ wk
        bo, b2v = f(inp['bo'][l]), f(inp['b2'][l])
        b1e = b2l @ w1f + f(inp['b1'][l])
        for db in range(DB):
            pv[:, 0 + db] = bq[db * 128:(db + 1) * 128]
            pv[:, 2 + db] = bk[db * 128:(db + 1) * 128]
            pv[:, 4 + db] = bo[db * 128:(db + 1) * 128]
            pv[:, 6 + db] = b2v[db * 128:(db + 1) * 128]
        for fb in range(8):
            pv[:, 8 + fb] = b1e[fb * 128:(fb + 1) * 128]
        p[f'pvec{l}'] = pv
        tab = f(inp['bias_table'][l])            # [2S-1, H]
        st = np.zeros((128, H, 1024), np.float32)
        for pp in range(128):
            hi = min(1024, pp + 2 * S - 1)
            st[pp, :, pp:hi] = tab[0:hi - pp, :].T
        # exp(bias) strip: probs = exp(scores) * expstrip (bias folded
        # multiplicatively, off the PE)
        p[f'strip{l}'] = np.exp(st).astype(ml_dtypes.bfloat16)
    fvec = np.zeros((128, 4), np.float32)
    for db in range(DB):
        fvec[:, 0 + db] = f(inp['fn_g'])[db * 128:(db + 1) * 128]
        fvec[:, 2 + db] = f(inp['fn_b'])[db * 128:(db + 1) * 128]
    p['fvec'] = fvec
    p['identf'] = np.eye(128, dtype=np.float32)
    p['onesd'] = np.full((128, 128), 1.0 / 256.0, np.float32)
    # vz slot template: per head slot h, Z-ones at cols [64*(h%2)+32, +32)
    vzt = np.zeros((128, 4, 4, 128), np.float32)
    for h in range(4):
        par = h % 2
        vzt[:, :, h, 64 * par + 32:64 * par + 64] = 1.0
    p['vztmpl'] = vzt.astype(ml_dtypes.bfloat16)
    return p


# ---------------------------------------------------------------- device build
def _build(repeat=1, upto='full'):
    nc = bacc.Bacc()
    din = {}

    def dinp(name, shape, dt=F32R):
        din[name] = nc.dram_tensor(name, list(shape), dt, kind='ExternalInput')
        return din[name]

    xA = dinp('xA', [BLOC, 128, S])
    xB = dinp('xB', [BLOC, 96, S])
    w1A = dinp('w1A', [128, D])
    w1B = dinp('w1B', [96, D])
    w2t = dinp('w2t', [128, DB, 5, D])
    cvec = dinp('cvec', [128, DB, 4], F32)
    peT = dinp('peT', [128, DB, S])
    identf = dinp('identf', [128, 128], F32)
    onesd = dinp('onesd', [128, 128], F32R)
    vztmpl = dinp('vztmpl', [128, 4, 4, 128], BF16)
    fvec = dinp('fvec', [128, 4], F32)
    for l in range(L):
        dinp(f'wqkvo{l}', [128, 4, DB, D])
        dinp(f'wff1{l}', [128, DB, FF])
        dinp(f'wff2{l}', [128, 8, D])
        dinp(f'bvb{l}', [128, D])
        dinp(f'pvec{l}', [128, 16], F32)
        dinp(f'strip{l}', [128, H, 1024], BF16)
    out = nc.dram_tensor('out', [BLOC, S, D], F32, kind='ExternalOutput')

    tc_cm = tile.TileContext(nc)
    tc = tc_cm.__enter__()
    cst = tc.alloc_tile_pool(name='cst', bufs=1)
    wp = tc.alloc_tile_pool(name='wp', bufs=2)
    wp1 = tc.alloc_tile_pool(name='wp1', bufs=1)
    ap = tc.alloc_tile_pool(name='ap', bufs=1)
    tp = tc.alloc_tile_pool(name='tp', bufs=2)
    tp1 = tc.alloc_tile_pool(name='tp1', bufs=1)
    vzp = tc.alloc_tile_pool(name='vzp', bufs=2)
    prp = tc.alloc_tile_pool(name='prp', bufs=2)
    h1p = tc.alloc_tile_pool(name='h1p', bufs=8)
    ps_mm = tc.alloc_tile_pool(name='ps_mm', bufs=2, space='PSUM')
    ps_sc = tc.alloc_tile_pool(name='ps_sc', bufs=2, space='PSUM')
    ps_cz = tc.alloc_tile_pool(name='ps_cz', bufs=2, space='PSUM')

    # ---- consts
    identf_s = cst.tile([128, 128], F32)
    nc.sync.dma_start(identf_s[:], identf[:])
    onesd_s = cst.tile([128, 128], F32R)
    nc.sync.dma_start(onesd_s[:], onesd[:])
    cvec_s = cst.tile([128, DB, 4], F32)
    nc.sync.dma_start(cvec_s[:], cvec[:])
    fvec_s = cst.tile([128, 4], F32)
    nc.sync.dma_start(fvec_s[:], fvec[:])
    peT_s = cst.tile([128, DB, S], F32R)
    nc.sync.dma_start(peT_s[:], peT[:])
    eps_s = cst.tile([128, 1], F32)
    nc.vector.memset(eps_s[:], EPS)
    z4_s = cst.tile([128, 4], F32)
    nc.vector.memset(z4_s[:], 0.0)
    w1A_s = cst.tile([128, D], F32R)
    nc.sync.dma_start(w1A_s[:], w1A[:])
    w1B_s = cst.tile([96, D], F32R)
    nc.sync.dma_start(w1B_s[:], w1B[:])
    w2t_s = cst.tile([128, DB, 5, D], F32R)
    nc.sync.dma_start(w2t_s[:], w2t[:])

    # vz slot templates (zeros + Z-ones). v columns are rewritten per use;
    # the static template regions persist across pool-slot reuse.
    for i in range(2):
        vzt_t = vzp.tile([128, 4, 4, 128], BF16, tag='vz', name=f'vzi{i}')
        nc.sync.dma_start(vzt_t[:], vztmpl[:])

    # persistent residual stream, feature-major [d mod 128, db, token]
    rt = ap.tile([128, DB, TOK], F32R)

    def emit_body(R):
        # ---------------- conv tokenizer (shifted inputs staged host-side)
        for b in range(BLOC):
            X4 = h1p.tile([128, 512], F32R, tag='h1', name=f'{R}x4_{b}')
            nc.sync.dma_start(X4[:], xA[b])
            X3 = h1p.tile([128, 512], F32R, tag='h1', name=f'{R}x3_{b}')
            nc.sync.dma_start(X3[0:96, :], xB[b])
            hp = tp.tile([128, DB, 516], F32R, tag='xn', name=f'{R}hp_{b}')
            for dc in range(DB):
                psc = ps_mm.tile([128, 512], F32, tag='mm', name=f'{R}c1_{b}_{dc}')
                nc.tensor.matmul(psc[:], w1A_s[:, dc * 128:(dc + 1) * 128],
                                 X4[:], start=True, stop=False,
                                 skip_group_check=True)
                nc.tensor.matmul(psc[:], w1B_s[:, dc * 128:(dc + 1) * 128],
                                 X3[0:96, :], start=False, stop=True,
                                 skip_group_check=True)
                nc.vector.tensor_copy(hp[:, dc, 0:2], z4_s[:, 0:2])
                nc.vector.tensor_copy(hp[:, dc, 514:516], z4_s[:, 0:2])
                nc.scalar.activation(hp[:, dc, 2:514], psc[:], AF.Gelu,
                                     bias=cvec_s[:, dc, 1:2],
                                     scale=cvec_s[:, dc, 0:1])
            for dc in range(DB):
                ps2 = ps_mm.tile([128, 512], F32, tag='mm', name=f'{R}c2_{b}_{dc}')
                for cb in range(DB):
                    for k in range(5):
                        nc.tensor.matmul(
                            ps2[:], w2t_s[:, cb, k, dc * 128:(dc + 1) * 128],
                            hp[:, cb, k:k + 512],
                            start=(cb == 0 and k == 0),
                            stop=(cb == 1 and k == 4), skip_group_check=True)
                tg = h1p.tile([128, 512], F32R, tag='h1', name=f'{R}tg_{b}_{dc}')
                nc.scalar.activation(tg[:], ps2[:], AF.Gelu,
                                     bias=cvec_s[:, dc, 3:4],
                                     scale=cvec_s[:, dc, 2:3])
                nc.vector.tensor_add(rt[:, dc, b * S:(b + 1) * S],
                                     tg[:], peT_s[:, dc, :])

        # ---------------- layernorm helper (feature-major, stats via ones-MM)
        def layernorm(xn_t, tag):
            outs = []
            for ic in range(IC):
                sl = slice(ic * 512, (ic + 1) * 512)
                sq = tp1.tile([128, DB, 512], F32R, tag='sq', name=f'{R}sq_{tag}{ic}')
                nc.scalar.activation(sq[:], rt[:, :, sl], AF.Square)
                mu_ps = ps_mm.tile([128, 512], F32, tag='mm', name=f'{R}mu_{tag}{ic}')
                s2_ps = ps_mm.tile([128, 512], F32, tag='mm', name=f'{R}s2_{tag}{ic}')
                for db in range(DB):
                    nc.tensor.matmul(mu_ps[:], onesd_s[:], rt[:, db, sl],
                                     start=(db == 0), stop=(db == 1),
                                     skip_group_check=True)
                for db in range(DB):
                    nc.tensor.matmul(s2_ps[:], onesd_s[:], sq[:, db, :],
                                     start=(db == 0), stop=(db == 1),
                                     skip_group_check=True)
                mu_sb = tp1.tile([128, 512], F32, tag='mu', name=f'{R}mu2_{tag}{ic}')
                nc.vector.tensor_copy(mu_sb[:], mu_ps[:])
                m2 = tp1.tile([128, 512], F32, tag='m2', name=f'{R}m2_{tag}{ic}')
                nc.scalar.activation(m2[:], mu_ps[:], AF.Square)
                var = tp1.tile([128, 512], F32, tag='var', name=f'{R}var_{tag}{ic}')
                nc.vector.tensor_sub(var[:], s2_ps[:], m2[:])
                sd = tp1.tile([128, 512], F32, tag='sd', name=f'{R}sd_{tag}{ic}')
                nc.scalar.activation(sd[:], var[:], AF.Sqrt, bias=eps_s[:])
                rstd = tp.tile([128, 512], F32, tag='rstd', name=f'{R}rs_{tag}{ic}')
                nc.vector.reciprocal_approx_fast(rstd[:], sd[:])
                nm = tp.tile([128, 512], F32, tag='nm', name=f'{R}nm_{tag}{ic}')
                nc.vector.scalar_tensor_tensor(nm[:], mu_sb[:], -1.0, rstd[:],
                                               ALU.mult, ALU.mult)
                if xn_t is not None:
                    for db in range(DB):
                        t1 = tp1.tile([128, 512], F32, tag='t1',
                                     name=f'{R}t1_{tag}{ic}{db}')
                        nc.gpsimd.tensor_tensor(t1[:], rt[:, db, sl], rstd[:],
                                                ALU.mult)
                        nc.vector.tensor_add(xn_t[:, db, sl], t1[:], nm[:])
                outs.append((rstd, nm))
            return outs

        # ---------------- transformer layers
        if upto == 'conv':
            return
        for l in range(L):
            wqkvo_s = wp.tile([128, 4, DB, D], F32R, tag='wqkvo', name=f'{R}wm{l}')
            nc.sync.dma_start(wqkvo_s[:], din[f'wqkvo{l}'][:])
            wff1_s = wp.tile([128, DB, FF], F32R, tag='wff1', name=f'{R}w1{l}')
            nc.sync.dma_start(wff1_s[:], din[f'wff1{l}'][:])
            wff2_s = wp1.tile([128, 8, D], F32R, tag='wff2', name=f'{R}w2{l}')
            nc.sync.dma_start(wff2_s[:], din[f'wff2{l}'][:])
            bvb_s = wp.tile([128, D], F32R, tag='bvb', name=f'{R}bv{l}')
            nc.sync.dma_start(bvb_s[:], din[f'bvb{l}'][:])
            pvec_s = wp.tile([128, 16], F32, tag='pvec', name=f'{R}pv{l}')
            nc.sync.dma_start(pvec_s[:], din[f'pvec{l}'][:])
            strip_s = wp1.tile([128, H, 1024], BF16, tag='strip', name=f'{R}st{l}')
            nc.sync.dma_start(strip_s[:], din[f'strip{l}'][:])

            # LN1 -> xn
            xn = tp.tile([128, DB, TOK], F32R, tag='xn', name=f'{R}xn{l}')
            layernorm(xn, f'a{l}')

            # q/k projections (feature-major, per-dout bias folded into evac)
            qT = ap.tile([128, DB, TOK], F32R, tag='qT', name=f'{R}qT{l}')
            kT = ap.tile([128, DB, TOK], F32R, tag='kT', name=f'{R}kT{l}')
            for mat, (dst, bc) in enumerate([(qT, 0), (kT, 2)]):
                for mb in range(DB):
                    for ic in range(IC):
                        sl = slice(ic * 512, (ic + 1) * 512)
                        ps = ps_mm.tile([128, 512], F32, tag='mm',
                                        name=f'{R}qk{l}{mat}{mb}{ic}')
                        for kb in range(DB):
                            nc.tensor.matmul(
                                ps[:],
                                wqkvo_s[:, mat, kb, mb * 128:(mb + 1) * 128],
                                xn[:, kb, sl], start=(kb == 0), stop=(kb == 1),
                                skip_group_check=True)
                        nc.vector.tensor_scalar(
                            dst[:, mb, sl], ps[:],
                            pvec_s[:, bc + mb:bc + mb + 1], None, ALU.add)

            if upto == 'qkvv':
                continue
            ctxT = ap.tile([128, DB, TOK], F32R, tag='ctxT', name=f'{R}cx{l}')
            bvb_r = bvb_s.rearrange('p (hb he pc) -> p hb he pc', hb=2, pc=64)
            for b in range(BLOC):
                # v projection (+bias fused into scatter), into padded vz slots
                vzs = []
                for hb in range(2):
                    vz = vzp.tile([128, 4, 4, 128], BF16, tag='vz',
                                  name=f'{R}vz{l}{b}{hb}')
                    vzs.append(vz)
                for jc in range(4):
                    vp = ps_mm.tile([128, 256], F32, tag='mm', name=f'{R}v{l}{b}{jc}')
                    for kb in range(DB):
                        nc.tensor.matmul(
                            vp[:],
                            xn[:, kb, b * S + jc * 128:b * S + (jc + 1) * 128],
                            wqkvo_s[:, 2, kb, :], start=(kb == 0), stop=(kb == 1),
                            skip_group_check=True)
                    vp_r = vp.rearrange('p (hb he pc) -> p hb he pc', hb=2, pc=64)
                    for hb in range(2):
                        vz_r = vzs[hb].rearrange(
                            'p jc (he two) m -> p jc he two m', two=2)
                        for par in range(2):
                            nc.vector.tensor_add(
                                vz_r[:, jc, :, par, 64 * par:64 * par + 32],
                                vp_r[:, hb, :, 32 * par:32 * par + 32],
                                bvb_r[:, hb, :, 32 * par:32 * par + 32])
                # attention per head-block: scoresT -> exp*bias -> ctx+Z matmuls
                for hb in range(2):
                    banks = [ps_cz.tile([128, 512], F32, tag='cz',
                                        name=f'{R}cb{l}{b}{hb}{pb}') for pb in range(2)]
                    for jc in range(4):
                        probs = []
                        for P in range(2):
                            scp = ps_sc.tile([128, 2, 512], F32, tag='sc',
                                             name=f'{R}sc{l}{b}{hb}{jc}{P}')
                            for q in range(2):
                                hh = 2 * P + q
                                nc.tensor.matmul(
                                    scp[:, q, :],
                                    kT[32 * hh:32 * hh + 32, hb,
                                       b * S + jc * 128:b * S + (jc + 1) * 128],
                                    qT[32 * hh:32 * hh + 32, hb, b * S:(b + 1) * S],
                                    start=True, stop=True,
                                    tile_position=(32 * hh, 0),
                                    skip_group_check=True)
                            pre = prp.tile([128, 2, 512], BF16, tag=f'e{P}',
                                           name=f'{R}pe{l}{b}{hb}{jc}{P}')
                            nc.scalar.activation(pre[:], scp[:], AF.Exp)
                            prm = prp.tile([128, 2, 512], BF16, tag=f'p{P}',
                                           name=f'{R}pr{l}{b}{hb}{jc}{P}')
                            nc.gpsimd.tensor_tensor(
                                prm[:], pre[:],
                                strip_s[:, 4 * hb + 2 * P:4 * hb + 2 * P + 2,
                                        511 - jc * 128:1023 - jc * 128],
                                ALU.mult)
                            probs.append(prm)
                        for hh in range(4):
                            nc.tensor.matmul(
                                banks[hh // 2][:], vzs[hb][:, jc, hh, :],
                                probs[hh // 2][:, hh % 2, :],
                                start=(jc == 0 and hh % 2 == 0),
                                stop=(jc == 3 and hh % 2 == 1),
                                skip_group_check=True)
                    # normalize + assemble ctxT rows [32h, +32)
                    for pb in range(2):
                        rec = tp1.tile([128, 512], F32, tag='rec',
                                      name=f'{R}rc{l}{b}{hb}{pb}')
                        nc.vector.reciprocal_approx_fast(rec[:], banks[pb][:])
                        zrec = tp1.tile([128, 512], F32, tag='zrec',
                                       name=f'{R}zr{l}{b}{hb}{pb}')
                        nc.sync.dma_start(zrec[0:32, :], rec[32:64, :])
                        nc.sync.dma_start(zrec[64:96, :], rec[96:128, :])
                        ctxn = tp1.tile([128, 512], F32R, tag='ctxn',
                                       name=f'{R}cn{l}{b}{hb}{pb}')
                        nc.vector.tensor_mul(ctxn[:], banks[pb][:], zrec[:])
                        base = 64 * pb
                        nc.sync.dma_start(
                            ctxT[base:base + 32, hb, b * S:(b + 1) * S],
                            ctxn[0:32, :])
                        nc.sync.dma_start(
                            ctxT[base + 32:base + 64, hb, b * S:(b + 1) * S],
                            ctxn[64:96, :])

            if upto == 'attn':
                continue
            # out-projection + residual (bias via fused scalar_tensor_tensor)
            for mb in range(DB):
                for ic in range(IC):
                    sl = slice(ic * 512, (ic + 1) * 512)
                    ps = ps_mm.tile([128, 512], F32, tag='mm',
                                    name=f'{R}op{l}{mb}{ic}')
                    for kb in range(DB):
                        nc.tensor.matmul(
                            ps[:], wqkvo_s[:, 3, kb, mb * 128:(mb + 1) * 128],
                            ctxT[:, kb, sl], start=(kb == 0), stop=(kb == 1),
                            skip_group_check=True)
                    nc.vector.scalar_tensor_tensor(
                        rt[:, mb, sl], ps[:], pvec_s[:, 4 + mb:5 + mb],
                        rt[:, mb, sl], ALU.add, ALU.add)

            # LN2 -> xn2
            xn2 = tp.tile([128, DB, TOK], F32R, tag='xn', name=f'{R}xn2_{l}')
            layernorm(xn2, f'f{l}')

            # FFN
            for ic in range(IC):
                sl = slice(ic * 512, (ic + 1) * 512)
                h1s = []
                for fb in range(8):
                    ps = ps_mm.tile([128, 512], F32, tag='mm', name=f'{R}h1{l}{ic}{fb}')
                    for kb in range(DB):
                        nc.tensor.matmul(
                            ps[:], wff1_s[:, kb, fb * 128:(fb + 1) * 128],
                            xn2[:, kb, sl], start=(kb == 0), stop=(kb == 1),
                            skip_group_check=True)
                    h1t = h1p.tile([128, 512], F32R, tag='h1', name=f'{R}h1t{l}{ic}{fb}')
                    nc.scalar.activation(h1t[:], ps[:], AF.Gelu,
                                         bias=pvec_s[:, 8 + fb:9 + fb])
                    h1s.append(h1t)
                for db in range(DB):
                    ps = ps_cz.tile([128, 512], F32, tag='cz', name=f'{R}h2{l}{ic}{db}')
                    for fb in range(8):
                        nc.tensor.matmul(
                            ps[:], wff2_s[:, fb, db * 128:(db + 1) * 128],
                            h1s[fb][:], start=(fb == 0), stop=(fb == 7),
                            skip_group_check=True)
                    nc.vector.scalar_tensor_tensor(
                        rt[:, db, sl], ps[:], pvec_s[:, 6 + db:7 + db],
                        rt[:, db, sl], ALU.add, ALU.add)

        if upto != 'full':
            sink = h1p.tile([128, 256], F32, tag='h1', name=f'{R}sink')
            nc.vector.tensor_copy(sink[:], rt[:, 0, 0:256])
            nc.sync.dma_start(out[0, 0:128, :], sink[:])
            return
        # ---------------- final LN (+affine) and transpose to token-major
        fin = tp.tile([128, DB, TOK], F32, tag='xn', name=f'{R}fin')
        stats = layernorm(None, 'fin')
        for ic in range(IC):
            sl = slice(ic * 512, (ic + 1) * 512)
            rstd, nm = stats[ic]
            for db in range(DB):
                rstd_g = tp1.tile([128, 512], F32, tag='rstd_g',
                                 name=f'{R}rg{ic}{db}')
                nc.vector.tensor_scalar(rstd_g[:], rstd[:],
                                        fvec_s[:, 0 + db:1 + db], None, ALU.mult)
                nm_gb = tp1.tile([128, 512], F32, tag='nm_gb', name=f'{R}ng{ic}{db}')
                nc.vector.tensor_scalar(nm_gb[:], nm[:],
                                        fvec_s[:, 0 + db:1 + db],
                                        fvec_s[:, 2 + db:3 + db],
                                        ALU.mult, ALU.add)
                t1 = tp1.tile([128, 512], F32, tag='t1', name=f'{R}ft1{ic}{db}')
                nc.gpsimd.tensor_tensor(t1[:], rt[:, db, sl], rstd_g[:], ALU.mult)
                nc.vector.tensor_add(fin[:, db, sl], t1[:], nm_gb[:])
        for b in range(BLOC):
            for jc in range(4):
                tc_sl = slice(b * S + jc * 128, b * S + (jc + 1) * 128)
                pst = ps_mm.tile([128, 256], F32, tag='mm', name=f'{R}tr{b}{jc}')
                for db in range(DB):
                    nc.tensor.transpose(pst[:, db * 128:(db + 1) * 128],
                                        fin[:, db, tc_sl], identf_s[:])
                osb = h1p.tile([128, 256], F32, tag='h1', name=f'{R}ot{b}{jc}')
                nc.vector.tensor_copy(osb[:], pst[:])
                nc.sync.dma_start(out[b, jc * 128:(jc + 1) * 128, :], osb[:])


    for _rep in range(repeat):
        emit_body(f'r{_rep}_')

    for pool in [ps_cz, ps_sc, ps_mm, h1p, prp, vzp, tp1, tp, ap, wp1, wp, cst]:
        pool.release()
    tc_cm.__exit__(None, None, None)
    nc.finalize()
    return nc


# ---------------------------------------------------------------- entry point
def kernel(**inputs):
    p = _prep(inputs)
    if 'nc' not in _CACHE:
        _CACHE['nc'] = _build()
    nc = _CACHE['nc']
    x = np.ascontiguousarray(np.asarray(inputs['x'], np.float32))
    # host-staged shifted conv inputs: xA rows = taps 0-3 of conv1 (with
    # left zero pad), xB rows = taps 4-6 (right zero pad)
    xA = np.zeros((B, 128, S), np.float32)
    xB = np.zeros((B, 96, S), np.float32)
    for kk in range(4):
        sh = 3 - kk
        xA[:, 32 * kk:32 * kk + 32, sh:S] = x[:, :, 0:S - sh]
    for j in range(3):
        sh = j + 1
        xB[:, 32 * j:32 * j + 32, 0:S - sh] = x[:, :, sh:S]
    in_maps = []
    for c in range(NCORES):
        m = dict(p)
        m['xA'] = np.ascontiguousarray(xA[c * BLOC:(c + 1) * BLOC])
        m['xB'] = np.ascontiguousarray(xB[c * BLOC:(c + 1) * BLOC])
        in_maps.append(m)
    res = run_bass_kernel_spmd(nc, in_maps, core_ids=list(range(NCORES)),
                               trace=TRACE)
    out = np.concatenate([r['out'] for r in res.results], axis=0)
    kernel.last_results = res
    return np.ascontiguousarray(out.astype(np.float32))

